# revision 20
# baseline (speedup 1.0000x reference)
"""Trainium2 Bass kernel for nn_MultiHeadAttention (Q.V^T attention variant).

Reference computation (B=2, S=2048, F=1024, H=16, D=64):
    q = query @ Wq + bq            -> [B,S,H,D]
    v = value @ Wv + bv            -> [B,S,H,D]
    score = einsum(bqhd,bkhd->bhqk)(q, v) / sqrt(D)
    align = softmax(score, -1)
    ctx = einsum(bhqk,bkhd->bqhd)(align, v)
    out = LN(concat([ctx, query], -1) @ Wfc + bfc) * gamma + beta

Sharding: 8 cores = 2 batches x 4 query-row chunks of 512 rows.

Every attention-side matmul runs in fp8e4 with MatmulPerfMode.DoubleRow
(2 stationary planes per matmul, 0.5 cycles/row):
  - q/v projections contract 2 feature-dtiles per matmul,
  - scores use a zero-weight second plane (contraction is only d=64),
  - context pairs two key-tiles as the two planes.
Scale management: query/value are pre-scaled x16 into fp8 on the host,
weights x256; projected q~/v~ land in SBUF as 32*(proj+bias) fp8 (bias added
during the PSUM->SBUF copy via per-partition tensor_scalar); the softmax exp
folds 1/(32*32*sqrt(D)) into its pre-scale; the context ones-column holds
32.0 so the softmax-denominator reciprocal undoes the value scaling for
free.  bv's contribution to fc is folded into an effective bfc on the host.
The fc matmul and its inputs (ctxT bf16, query bf16) stay bf16 for accuracy.

Scheduling: projections run vT -> q -> V so attention pair 0 starts as soon
as the first AllGather rank lands (the tile framework tracks scatter regions
per rank, so scores(kt) only wait for rank kt//4).  The fc query-half runs
during the attention window into an SBUF accumulator (dedicated psum
buffer); the tail is only the fc context-half + LayerNorm.
"""

import numpy as np
import ml_dtypes

import concourse.bass as bass
import concourse.tile as tile
from concourse import bacc, mybir
from concourse.bass_utils import run_bass_kernel_spmd

BF16 = mybir.dt.bfloat16
F32 = mybir.dt.float32
FP8 = mybir.dt.float8e4
NP_BF16 = ml_dtypes.bfloat16
NP_FP8 = ml_dtypes.float8_e4m3

B, S, F, H, D = 2, 2048, 1024, 16, 64
NCORES = 8
RPC = 512            # query rows per core
CHUNKS = 4           # row chunks per batch (= cores per batch group)
KEYS = S             # 2048 keys per batch
NKT = KEYS // 128    # 16 key tiles
NDT = F // 128       # 8 feature tiles
NPAIR = H // 2       # 8 head pairs
EPS = 1e-5

SX = 16.0            # raw query/value scale into fp8
SW = 256.0           # weight scale into fp8
SP = 32.0            # projected q~/v~ scale in fp8
COPY_SCALE = SP / (SX * SW)          # psum -> fp8 copy scale (1/128)
EXP_SCALE = 1.0 / (SP * SP * np.sqrt(D))   # fold 1/sqrt(D) and q/v scales
FP8_MAX = 240.0      # IEEE float8_e4m3 max finite

# AllGather payload layout (fp8 elements):
#   region A: vT chunk as [8 dtile, 128, 512]
#   region B: V  chunk as [8 (keytile,half), 128, 520]  (520 = 8 heads x 65)
A_ELEMS = NDT * 128 * 512           # 524288
B_BLOCK = 128 * 520                 # 66560
B_ELEMS = 8 * B_BLOCK               # 532480

DR = mybir.MatmulPerfMode.DoubleRow

NO_COLL = False
NO_COLL_FREE = False  # timing-only: omit the gather traffic entirely
APPLY_GB = True   # apply gamma/beta in the LN epilogue (skippable when ==1/0)
USE_PB = True     # gpsimd partition_broadcast for the softmax denominators


def _build_kernel():
    nc = bacc.Bacc(
        "TRN2",
        target_bir_lowering=False,
        debug=False,
        enable_asserts=False,
        num_devices=NCORES,
    )

    qT_d = nc.dram_tensor("qT", [F, RPC], BF16, kind="ExternalInput")
    q8_d = nc.dram_tensor("q8", [F, RPC], FP8, kind="ExternalInput")
    v8_d = nc.dram_tensor("v8", [F, RPC], FP8, kind="ExternalInput")
    wq8_d = nc.dram_tensor("wq8", [F, F], FP8, kind="ExternalInput")
    wv8_d = nc.dram_tensor("wv8", [F, F], FP8, kind="ExternalInput")
    bqc_d = nc.dram_tensor("bqc", [128, NDT], F32, kind="ExternalInput")
    bvc_d = nc.dram_tensor("bvc", [128, NDT], F32, kind="ExternalInput")
    wfc_d = nc.dram_tensor("wfc", [2 * F + 1, F], BF16, kind="ExternalInput")
    gam_d = nc.dram_tensor("gam", [1, F], F32, kind="ExternalInput")
    bet_d = nc.dram_tensor("bet", [1, F], F32, kind="ExternalInput")
    out_d = nc.dram_tensor("out", [RPC, F], F32, kind="ExternalOutput")

    with tile.TileContext(nc) as tc:
        _kernel_body(tc, qT_d, q8_d, v8_d, wq8_d, wv8_d, bqc_d, bvc_d,
                     wfc_d, gam_d, bet_d, out_d)

    nc.compile()
    return nc


def _bcast_row_ap(t, n):
    """AP reading DRAM row tensor [1, n] broadcast to 128 partitions."""
    ap = t.ap()
    return bass.AP(tensor=ap.tensor, offset=ap.offset, ap=[[0, 128], [1, n]])


def _dr(base, plane_stride):
    """Insert the 2-wide DoubleRow plane dim into a [part, n] AP slice."""
    assert len(base.ap) == 2, base.ap
    return bass.AP(tensor=base.tensor, offset=base.offset,
                   ap=[base.ap[0], [plane_stride, 2], base.ap[1]])


def _kernel_body(tc, qT_d, q8_d, v8_d, wq8_d, wv8_d, bqc_d, bvc_d,
                 wfc_d, gam_d, bet_d, out_d):
    nc = tc.nc
    Exp = mybir.ActivationFunctionType.Exp
    Sqrt = mybir.ActivationFunctionType.Sqrt
    Copy = mybir.ActivationFunctionType.Copy
    Square = mybir.ActivationFunctionType.Square
    mult = mybir.AluOpType.mult
    addop = mybir.AluOpType.add
    subop = mybir.AluOpType.subtract

    import contextlib
    ctx = contextlib.ExitStack()
    with ctx:
        persist = ctx.enter_context(tc.tile_pool(name="persist", bufs=1))
        small = ctx.enter_context(tc.tile_pool(name="small", bufs=2))
        bcpool = ctx.enter_context(tc.tile_pool(name="bcpool", bufs=2))
        lnp = ctx.enter_context(tc.tile_pool(name="lnp", bufs=2))
        pps = ctx.enter_context(tc.tile_pool(name="pps", bufs=2, space="PSUM"))
        pctx = ctx.enter_context(tc.tile_pool(name="pctx", bufs=1, space="PSUM"))
        pfc = ctx.enter_context(tc.tile_pool(name="pfc", bufs=1, space="PSUM"))
        dram = ctx.enter_context(tc.tile_pool(name="dram", bufs=1, space="DRAM"))

        # ---- persistent SBUF buffers ----
        vraw8 = persist.tile([128, NDT * RPC], FP8)      # value^T chunk, fp8
        qraw8 = persist.tile([128, NDT * RPC], FP8)      # query^T chunk, fp8
        qTin = persist.tile([128, NDT * RPC], BF16)      # query^T chunk (for fc)
        qT8 = persist.tile([128, NDT * RPC], FP8)        # projected q~, fp8
        wv_all = persist.tile([128, NDT * F], FP8)       # all Wv row-dtiles
        wq_all = persist.tile([128, NDT * F], FP8)       # all Wq row-dtiles
        vT8s = persist.tile([128, NDT * RPC], FP8)       # own v~^T chunk (AG A)
        V8s = persist.tile([128, 8 * 520], FP8)          # own V chunk (AG B)
        vT8f = persist.tile([128, (NDT + 1) * KEYS], FP8)  # v~^T all keys + zero blk
        V8f = persist.tile([128, NKT * 1040], FP8)       # V all keys, 65-col blocks
        pt8 = persist.tile([128, 2 * NKT * 1024], FP8)   # exp(scores^T), fp8, 2 pair-parities
        ctxT = persist.tile([128, NPAIR * RPC], BF16)    # normalized context^T
        wfc_sb = persist.tile([128, 2 * NDT * 1024], BF16)  # all fc weights
        fcq = persist.tile([128, 4 * 1024], F32)         # fc query-half partials
        ones64 = persist.tile([1, 64], BF16)
        ones_bf = persist.tile([1, 128], BF16)
        bqc = persist.tile([128, NDT], F32)
        bvc = persist.tile([128, NDT], F32)
        bfc_sb = persist.tile([1, F], BF16)
        eps_sb = persist.tile([128, 1], F32)
        if APPLY_GB:
            gamma_bc = persist.tile([128, F], F32)
            beta_bc = persist.tile([128, F], F32)

        ag_inA = [dram.tile([A_ELEMS // 2], FP8, name=f"agiA{i}")
                  for i in range(2)]
        ag_outA = [dram.tile([CHUNKS, A_ELEMS // 2], FP8, name=f"agoA{i}")
                   for i in range(2)]
        ag_inB = [dram.tile([B_ELEMS // 2], FP8, name=f"agiB{i}")
                  for i in range(2)]
        ag_outB = [dram.tile([CHUNKS, B_ELEMS // 2], FP8, name=f"agoB{i}")
                   for i in range(2)]

        nc.vector.memset(ones64[:, :], 1.0)
        nc.vector.memset(ones_bf[:, :], 1.0)
        nc.vector.memset(eps_sb[:, :], EPS)
        # zero weight block for the scores' second DoubleRow plane
        nc.gpsimd.memset(vT8f[:, NDT * KEYS:(NDT + 1) * KEYS], 0.0)
        # ones columns staged in V8s (=SP so the denominator reciprocal undoes
        # the V scale); the gather then carries them into every rank's V8f
        nc.gpsimd.memset(
            V8s[:, :].rearrange("p (b h e) -> p b h e", b=8, e=65)[:, :, :, 64:65],
            SP)

        nc.sync.dma_start(out=bvc[:, :], in_=bvc_d[0:128, :])
        nc.sync.dma_start(out=bqc[:, :], in_=bqc_d[0:128, :])

        # batched input loads, ordered so the vT pass starts earliest
        def load_w_half(dst, w_d, hf):
            nc.sync.dma_start(
                out=dst[:, :].rearrange("p (c f) -> p c f", c=NDT)[
                    :, hf * 4:(hf + 1) * 4, :],
                in_=w_d[hf * 512:(hf + 1) * 512, :].rearrange(
                    "(c p) f -> p c f", p=128))

        def load_x_half(dst3, x_d, hf):
            nc.sync.dma_start(
                out=dst3[:, hf * 4:(hf + 1) * 4, :],
                in_=x_d[hf * 512:(hf + 1) * 512, :].rearrange(
                    "(c p) n -> p c n", p=128))

        vraw3 = vraw8[:, :].rearrange("p (c n) -> p c n", c=NDT)
        qraw3 = qraw8[:, :].rearrange("p (c n) -> p c n", c=NDT)
        load_w_half(wv_all, wv8_d, 0)
        load_x_half(vraw3, v8_d, 0)
        load_w_half(wv_all, wv8_d, 1)
        load_x_half(vraw3, v8_d, 1)
        load_w_half(wq_all, wq8_d, 0)
        load_x_half(qraw3, q8_d, 0)
        load_w_half(wq_all, wq8_d, 1)
        load_x_half(qraw3, q8_d, 1)

        wv3 = wv_all[:, :].rearrange("p (c f) -> p c f", c=NDT)
        wq3 = wq_all[:, :].rearrange("p (c f) -> p c f", c=NDT)

        def alloc_octet(nm):
            pss = []
            for i in range(2):
                big = pps.tile([128, 1024], F32, tag="ps", name=f"{nm}{i}")
                pss.append(big[:, 0:512])
                pss.append(big[:, 512:1024])
            big = pctx.tile([128, 1024], F32, tag="cps", name=f"{nm}c")
            pss.append(big[:, 0:512])
            pss.append(big[:, 512:1024])
            big = pfc.tile([128, 1024], F32, tag="pfc", name=f"{nm}f")
            pss.append(big[:, 0:512])
            pss.append(big[:, 512:1024])
            return pss

        def proj_matmuls(nm, w3v, x3):
            pss = alloc_octet(nm)
            for c in range(4):
                for m in range(NDT):
                    nc.tensor.matmul(pss[m][:, :],
                                     w3v[:, 2 * c:2 * c + 2, m * 128:(m + 1) * 128],
                                     x3[:, 2 * c:2 * c + 2, :],
                                     start=(c == 0), stop=(c == 3),
                                     perf_mode=DR)
            return pss

        def proj_copies(pss, dst, bias, ms):
            with nc.allow_low_precision(reason="fp8 attention operands"):
                for m in ms:
                    nc.vector.tensor_scalar(dst[:, m * RPC:(m + 1) * RPC],
                                            pss[m][:, :], COPY_SCALE,
                                            bias[:, m:m + 1], op0=mult, op1=addop)

        def all_gather(in_ap, out_ap):
            if NO_COLL_FREE:
                nc.sync.dma_start(out=out_ap[0], in_=in_ap)
            elif NO_COLL:
                for r in range(CHUNKS):
                    nc.sync.dma_start(out=out_ap[r], in_=in_ap)
            else:
                nc.gpsimd.collective_compute(
                    "AllGather",
                    mybir.AluOpType.bypass,
                    replica_groups=[[0, 1, 2, 3], [4, 5, 6, 7]],
                    ins=[in_ap],
                    outs=[out_ap],
                )

        def gather_A(sub):
            # sub 0: dtiles 0-3, sub 1: dtiles 4-7
            nc.sync.dma_start(
                out=ag_inA[sub][:].rearrange("(t p n) -> p t n", p=128, t=4),
                in_=vT8s[:, :].rearrange("p (t n) -> p t n", t=NDT)[
                    :, sub * 4:(sub + 1) * 4, :])
            all_gather(ag_inA[sub][:], ag_outA[sub][:, :])
            for r in range(CHUNKS):
                nc.gpsimd.dma_start(
                    out=vT8f[:, :].rearrange("p (t n) -> p t n", t=NDT + 1)[
                        :, sub * 4:(sub + 1) * 4, r * RPC:(r + 1) * RPC],
                    in_=ag_outA[sub][r, :].rearrange(
                        "(t p n) -> p t n", p=128, t=4))

        def gather_B(sub):
            # sub 0: key-tiles {4r, 4r+1}, sub 1: {4r+2, 4r+3} of every rank
            nc.sync.dma_start(
                out=ag_inB[sub][:].rearrange("(b p n) -> p b n", p=128, b=4),
                in_=V8s[:, :].rearrange("p (b n) -> p b n", b=8)[
                    :, sub * 4:(sub + 1) * 4, :])
            all_gather(ag_inB[sub][:], ag_outB[sub][:, :])
            for r in range(CHUNKS):
                nc.gpsimd.dma_start(
                    out=V8f[:, :].rearrange("p (k h c) -> p k h c", k=NKT, h=2)[
                        :, r * 4 + sub * 2:r * 4 + sub * 2 + 2, :, :],
                    in_=ag_outB[sub][r, :].rearrange(
                        "(k h p n) -> p k h n", p=128, k=2, h=2))

        # ---- v~^T projection; sub-gather A while q projects ----
        pssT = proj_matmuls("vt", wv3, vraw3)
        proj_copies(pssT, vT8s, bvc, range(0, 4))
        gather_A(0)

        # ---- q~ projection ----
        pssQ = proj_matmuls("qp", wq3, qraw3)
        proj_copies(pssQ, qT8, bqc, range(0, 2))
        proj_copies(pssT, vT8s, bvc, range(4, 8))
        proj_copies(pssQ, qT8, bqc, range(2, 8))

        # ---- V projection in two half-octets on the cps/pfc psums only,
        # so the attention ps ring is never gated on the V chain ----
        def v_half(sub):
            pss = []
            big = pctx.tile([128, 1024], F32, tag="cps", name=f"vv{sub}c")
            pss.append(big[:, 0:512])
            pss.append(big[:, 512:1024])
            big = pfc.tile([128, 1024], F32, tag="pfc", name=f"vv{sub}f")
            pss.append(big[:, 0:512])
            pss.append(big[:, 512:1024])
            for c in range(4):
                for t in (2 * sub, 2 * sub + 1):
                    for half in range(2):
                        nc.tensor.matmul(
                            pss[(t - 2 * sub) * 2 + half][:, :],
                            vraw3[:, 2 * c:2 * c + 2, t * 128:(t + 1) * 128],
                            wv3[:, 2 * c:2 * c + 2, half * 512:(half + 1) * 512],
                            start=(c == 0), stop=(c == 3), perf_mode=DR)
            with nc.allow_low_precision(reason="fp8 attention operands"):
                for t in (2 * sub, 2 * sub + 1):
                    for half in range(2):
                        b = t * 2 + half
                        nc.vector.tensor_scalar(
                            V8s[:, b * 520:(b + 1) * 520].rearrange(
                                "p (h e) -> p h e", e=65)[:, :, 0:64],
                            pss[(t - 2 * sub) * 2 + half][:, :].rearrange(
                                "p (h d) -> p h d", d=64),
                            COPY_SCALE, None, op0=mult)
            gather_B(sub)

        v_half(0)
        v_half(1)
        # A2 feeds only pairs 4-7; gate it behind the B gathers and the fc
        # weight loads so those own the DMA queue first (bypass rewrites one
        # byte of the stage source with its own value, adding only the read
        # edge on the last wfc block)
        nc.vector.tensor_tensor(vT8s[0:1, 4 * RPC:4 * RPC + 1],
                                vT8s[0:1, 4 * RPC:4 * RPC + 1],
                                wfc_sb[0:1, 15 * 1024:15 * 1024 + 1],
                                op=mybir.AluOpType.bypass)
        gather_A(1)

        # ---- attention, one head pair at a time ----
        def normalize_pair(p, cps):
            bcs = bcpool.tile([64, 1024], F32, tag="bcs")
            if USE_PB:
                rec = small.tile([1, 1024], F32, tag="rec")
                nc.vector.reciprocal(rec[:, :], cps[64:65, :])
                nc.gpsimd.partition_broadcast(bcs[:, :], rec[:, :], channels=64)
            else:
                rec = small.tile([1, 1024], BF16, tag="rec")
                with nc.allow_low_precision(reason="softmax denom recip bf16"):
                    nc.vector.reciprocal(rec[:, :], cps[64:65, :])
                bc = pps.tile([128, 1024], F32, tag="ps", name="bc")
                nc.tensor.matmul(bc[0:64, 0:512], ones64[:, :], rec[:, 0:512],
                                 start=True, stop=True)
                nc.tensor.matmul(bc[0:64, 512:1024], ones64[:, :],
                                 rec[:, 512:1024], start=True, stop=True)
                nc.vector.tensor_copy(bcs[:, :], bc[0:64, :])
            nc.vector.tensor_tensor(
                ctxT[0:64, p * RPC:(p + 1) * RPC],
                cps[0:64, 0:512], bcs[:, 0:512], op=mult)
            nc.vector.tensor_tensor(
                ctxT[64:128, p * RPC:(p + 1) * RPC],
                cps[0:64, 512:1024], bcs[:, 512:1024], op=mult)

        def fc_query_group(m):
            # accumulate the query half of fc for row-tile m into fcq (SBUF)
            pf = pfc.tile([128, 1024], F32, tag="pfc", name="pf")
            for kc in range(NDT, 2 * NDT):
                cblk = (kc - NDT) * RPC
                wcol = kc * 1024
                for n in range(2):
                    nc.tensor.matmul(
                        pf[:, n * 512:(n + 1) * 512],
                        qTin[:, cblk + m * 128:cblk + (m + 1) * 128],
                        wfc_sb[:, wcol + n * 512:wcol + (n + 1) * 512],
                        start=(kc == NDT), stop=(kc == 2 * NDT - 1))
            nc.vector.tensor_copy(fcq[:, m * 1024:(m + 1) * 1024], pf[:, :])

        def ctx_block(cp, kt):
            # context^T for pair cp over the (kt-1, kt) key-tile pair
            par = (cp % 2) * NKT * 1024
            cps = ctx_cps[cp]
            vcol = (kt - 1) * 1040
            pcol = par + (kt - 1) * 1024
            nc.tensor.matmul(
                cps[:, 0:512],
                _dr(V8f[:, vcol + (2 * cp) * 65:vcol + (2 * cp) * 65 + 65],
                    1040),
                _dr(pt8[:, pcol:pcol + 512], 1024),
                start=(kt == 1), stop=(kt == NKT - 1), perf_mode=DR)
            nc.tensor.matmul(
                cps[:, 512:1024],
                _dr(V8f[:, vcol + (2 * cp + 1) * 65:vcol + (2 * cp + 1) * 65 + 65],
                    1040),
                _dr(pt8[:, pcol + 512:pcol + 1024], 1024),
                start=(kt == 1), stop=(kt == NKT - 1), perf_mode=DR)

        ctx_cps = {}

        def attn_pair(p):
            # scores/exp for pair p; ctx matmuls lag one pair so they never
            # park in the PE wait queue before the V gather lands
            par = (p % 2) * NKT * 1024
            zoff = NDT * KEYS  # zero block start in vT8f
            for kt in range(NKT):
                ps = pps.tile([128, 1024], F32, tag="ps")
                col = p * KEYS + kt * 128
                nc.tensor.matmul(
                    ps[:, 0:512],
                    _dr(vT8f[0:64, col:col + 128], zoff - p * KEYS),
                    _dr(qT8[0:64, p * RPC:(p + 1) * RPC], 0),
                    start=True, stop=True, perf_mode=DR)
                nc.tensor.matmul(
                    ps[:, 512:1024],
                    _dr(vT8f[64:128, col:col + 128], zoff - p * KEYS),
                    _dr(qT8[64:128, p * RPC:(p + 1) * RPC], 0),
                    start=True, stop=True, perf_mode=DR)
                nc.scalar.activation(pt8[:, par + kt * 1024:par + (kt + 1) * 1024],
                                     ps[:, :], Exp, scale=EXP_SCALE)
                if p >= 1:
                    if kt == 1:
                        if p >= 2:
                            normalize_pair(p - 2, ctx_cps[p - 2])
                        ctx_cps[p - 1] = pctx.tile([65, 1024], F32, tag="cps",
                                                   name="cps")
                        if p == NPAIR - 1:
                            # last pair's own ctx runs un-lagged on the pfc
                            # ring (V8f has long been gathered by now)
                            ctx_cps[p] = pfc.tile([65, 1024], F32, tag="pfc",
                                                  name="cps7")
                    if kt % 2 == 1:
                        ctx_block(p - 1, kt)
                        if p == NPAIR - 1:
                            ctx_block(p, kt)

        def bulk_loads():
            # WAR gate: a dummy write into each destination reads a region
            # the last sub-gather's scatter writes, so these big loads only
            # hit the DMA queue after the startup gather chain is through.
            gate = V8f[0:1, 14 * 1040:14 * 1040 + 1]  # scatter_B(1) rank 3
            with nc.allow_low_precision(reason="scheduling gate dummies"):
                nc.vector.tensor_copy(qTin[0:1, 0:1], gate)
                for qt in range(4):
                    nc.vector.tensor_copy(
                        wfc_sb[0:1, qt * 4 * 1024:qt * 4 * 1024 + 1], gate)
            nc.sync.dma_start(
                out=qTin[:, :].rearrange("p (c n) -> p c n", c=NDT),
                in_=qT_d[:, :].rearrange("(c p) n -> p c n", p=128))
            for qt in range(4):
                nc.sync.dma_start(
                    out=wfc_sb[:, :].rearrange("p (c f) -> p c f", c=2 * NDT)[
                        :, qt * 4:(qt + 1) * 4, :],
                    in_=wfc_d[qt * 512:(qt + 1) * 512, :].rearrange(
                        "(c p) f -> p c f", p=128))
            nc.sync.dma_start(out=bfc_sb[:, :],
                              in_=wfc_d[2 * F:2 * F + 1, :])
            if APPLY_GB:
                nc.sync.dma_start(out=gamma_bc[:, :],
                                  in_=_bcast_row_ap(gam_d, F))
                nc.sync.dma_start(out=beta_bc[:, :],
                                  in_=_bcast_row_ap(bet_d, F))

        def fc_ctx_partial(m):
            # fold the kc0-3 context half of fc for row-tile m into fcq
            pf = pfc.tile([128, 1024], F32, tag="pfc", name="pcp")
            for kc in range(4):
                cblk = kc * RPC
                wcol = kc * 1024
                for n in range(2):
                    nc.tensor.matmul(
                        pf[:, n * 512:(n + 1) * 512],
                        ctxT[:, cblk + m * 128:cblk + (m + 1) * 128],
                        wfc_sb[:, wcol + n * 512:wcol + (n + 1) * 512],
                        start=(kc == 0), stop=(kc == 3))
            nc.vector.tensor_tensor(fcq[:, m * 1024:(m + 1) * 1024],
                                    fcq[:, m * 1024:(m + 1) * 1024],
                                    pf[:, :], op=addop)

        for p in range(NPAIR):
            attn_pair(p)
            if p == 0:
                bulk_loads()
            if 2 <= p <= 5:
                fc_query_group(p - 2)
            if p == 5:
                fc_ctx_partial(0)
                fc_ctx_partial(1)
            if p == 6:
                fc_ctx_partial(2)
                fc_ctx_partial(3)
        # trailer: only the last pair's normalize remains
        normalize_pair(NPAIR - 1, ctx_cps[NPAIR - 1])

        # ---- fc context-half tail (kc4-7) + LayerNorm ----
        # kc4-6 for every row-tile first (their ctxT pairs are normalized
        # well before the attention trailer); kc7 + bias + the LN chain per
        # row-tile after, so the last normalize overlaps fc matmuls.
        pssm = []
        bigA = pps.tile([128, 1024], F32, tag="ps", name="fcA")
        bigB = pctx.tile([128, 1024], F32, tag="cps", name="fcB")
        bigA2 = pps.tile([128, 1024], F32, tag="ps", name="fcA2")
        bigB2 = pfc.tile([128, 1024], F32, tag="pfc", name="fcB2")
        for big in (bigA, bigB, bigA2, bigB2):
            pssm.append(big)
        for m in range(4):
            for kc in range(4, 7):
                cblk = kc * RPC
                wcol = kc * 1024
                for n in range(2):
                    nc.tensor.matmul(
                        pssm[m][:, n * 512:(n + 1) * 512],
                        ctxT[:, cblk + m * 128:cblk + (m + 1) * 128],
                        wfc_sb[:, wcol + n * 512:wcol + (n + 1) * 512],
                        start=(kc == 4), stop=False)
        for m in range(4):
            big = pssm[m]
            for n in range(2):
                nc.tensor.matmul(
                    big[:, n * 512:(n + 1) * 512],
                    ctxT[:, 7 * RPC + m * 128:7 * RPC + (m + 1) * 128],
                    wfc_sb[:, 7 * 1024 + n * 512:7 * 1024 + (n + 1) * 512],
                    start=False, stop=False)
                nc.tensor.matmul(big[:, n * 512:(n + 1) * 512],
                                 ones_bf[:, :],
                                 bfc_sb[:, n * 512:(n + 1) * 512],
                                 start=False, stop=True)
            tsb = lnp.tile([128, 1024], F32, tag="tsb", name="tsb")
            for n in range(2):
                nc.vector.tensor_tensor(
                    tsb[:, n * 512:(n + 1) * 512], big[:, n * 512:(n + 1) * 512],
                    fcq[:, m * 1024 + n * 512:m * 1024 + (n + 1) * 512],
                    op=addop)
            dump = lnp.tile([128, 1024], F32, tag="dump", name="dump")
            dump2 = lnp.tile([128, 1024], F32, tag="dump", name="dump2")
            ssum = small.tile([128, 1], F32, tag="ssum", name="ssum")
            sqs = small.tile([128, 1], F32, tag="sqs", name="sqs")
            nc.scalar.activation(dump[:, :], tsb[:, :], Copy,
                                 accum_out=ssum[:, :])
            nc.scalar.activation(dump2[:, :], tsb[:, :], Square,
                                 accum_out=sqs[:, :])
            mean = small.tile([128, 1], F32, tag="mean", name="mean")
            nc.vector.tensor_scalar(mean[:, :], ssum[:, :], 1.0 / F, None,
                                    op0=mult)
            ex2 = small.tile([128, 1], F32, tag="ex2", name="ex2")
            nc.vector.tensor_scalar(ex2[:, :], sqs[:, :], 1.0 / F, None,
                                    op0=mult)
            msq = small.tile([128, 1], F32, tag="msq", name="msq")
            nc.vector.tensor_tensor(msq[:, :], mean[:, :], mean[:, :], op=mult)
            var = small.tile([128, 1], F32, tag="var", name="var")
            nc.vector.tensor_tensor(var[:, :], ex2[:, :], msq[:, :], op=subop)
            sd = small.tile([128, 1], F32, tag="sd", name="sd")
            nc.scalar.activation(sd[:, :], var[:, :], Sqrt, bias=eps_sb[:, :])
            rstd = small.tile([128, 1], F32, tag="rstd", name="rstd")
            nc.vector.reciprocal(rstd[:, :], sd[:, :])
            nmr = small.tile([128, 1], F32, tag="nmr", name="nmr")
            nc.vector.tensor_scalar(nmr[:, :], mean[:, :], rstd[:, :], -1.0,
                                    op0=mult, op1=mult)
            outt = lnp.tile([128, F], F32, tag="dump", name="outt")
            for n in range(2):
                sl = slice(n * 512, (n + 1) * 512)
                if APPLY_GB:
                    t1 = lnp.tile([128, 512], F32, tag="t1", name="t1")
                    nc.vector.tensor_scalar(t1[:, :], tsb[:, sl],
                                            rstd[:, :], nmr[:, :],
                                            op0=mult, op1=addop)
                    t2 = lnp.tile([128, 512], F32, tag="t1", name="t2")
                    nc.vector.tensor_tensor(t2[:, :], t1[:, :],
                                            gamma_bc[:, sl], op=mult)
                    nc.vector.tensor_tensor(outt[:, sl], t2[:, :],
                                            beta_bc[:, sl], op=addop)
                else:
                    nc.vector.tensor_scalar(outt[:, sl], tsb[:, sl],
                                            rstd[:, :], nmr[:, :],
                                            op0=mult, op1=addop)
                nc.sync.dma_start(out=out_d[m * 128:(m + 1) * 128, sl],
                                  in_=outt[:, sl])


_NC_CACHE = {}


def _get_nc():
    key = (APPLY_GB, NO_COLL, NO_COLL_FREE, USE_PB)
    if key not in _NC_CACHE:
        _NC_CACHE[key] = _build_kernel()
    return _NC_CACHE[key]


def _to_fp8(x):
    return np.clip(x, -FP8_MAX, FP8_MAX).astype(NP_FP8)


def _prep_inputs(query, value, Wq, bq, Wv, bv, Wfc, bfc, gamma, beta):
    wq8 = np.ascontiguousarray(_to_fp8(Wq * SW))
    wv8 = np.ascontiguousarray(_to_fp8(Wv * SW))
    bqc = np.ascontiguousarray((bq * SP).reshape(NDT, 128).T).astype(np.float32)
    bvc = np.ascontiguousarray((bv * SP).reshape(NDT, 128).T).astype(np.float32)
    # fold bv's contribution through the context half of Wfc into the bias
    bfc_eff = bfc + bv @ Wfc[:F, :]
    wfc_ext = np.ascontiguousarray(
        np.concatenate([Wfc, bfc_eff[None, :]], axis=0)).astype(NP_BF16)
    gam = np.ascontiguousarray(gamma[None, :]).astype(np.float32)
    bet = np.ascontiguousarray(beta[None, :]).astype(np.float32)

    in_maps = []
    for c in range(NCORES):
        b, r = c // CHUNKS, (c % CHUNKS) * RPC
        qT = np.ascontiguousarray(query[b, r:r + RPC, :].T)
        vT = np.ascontiguousarray(value[b, r:r + RPC, :].T)
        in_maps.append({
            "qT": qT.astype(NP_BF16),
            "q8": _to_fp8(qT * SX),
            "v8": _to_fp8(vT * SX),
            "wq8": wq8, "wv8": wv8, "bqc": bqc, "bvc": bvc,
            "wfc": wfc_ext, "gam": gam, "bet": bet,
        })
    return in_maps


def run_on_hw(in_maps, **kwargs):
    nc = _get_nc()
    return run_bass_kernel_spmd(nc, in_maps, list(range(NCORES)), **kwargs)


def kernel(query, value, Wq, bq, Wv, bv, Wfc, bfc, gamma, beta):
    global APPLY_GB
    APPLY_GB = not (np.all(np.asarray(gamma, np.float32) == 1.0)
                    and np.all(np.asarray(beta, np.float32) == 0.0))
    query = np.asarray(query, dtype=np.float32)
    value = np.asarray(value, dtype=np.float32)
    in_maps = _prep_inputs(query, value,
                           np.asarray(Wq, np.float32), np.asarray(bq, np.float32),
                           np.asarray(Wv, np.float32), np.asarray(bv, np.float32),
                           np.asarray(Wfc, np.float32), np.asarray(bfc, np.float32),
                           np.asarray(gamma, np.float32), np.asarray(beta, np.float32))
    res = run_on_hw(in_maps)
    out = np.empty((B, S, F), np.float32)
    for c in range(NCORES):
        b, r = c // CHUNKS, (c % CHUNKS) * RPC
        out[b, r:r + RPC, :] = res.results[c]["out"]
    return out


# revision 21
# speedup vs baseline: 1.0138x; 1.0138x over previous
"""Trainium2 Bass kernel for nn_MultiHeadAttention (Q.V^T attention variant).

Reference computation (B=2, S=2048, F=1024, H=16, D=64):
    q = query @ Wq + bq            -> [B,S,H,D]
    v = value @ Wv + bv            -> [B,S,H,D]
    score = einsum(bqhd,bkhd->bhqk)(q, v) / sqrt(D)
    align = softmax(score, -1)
    ctx = einsum(bhqk,bkhd->bqhd)(align, v)
    out = LN(concat([ctx, query], -1) @ Wfc + bfc) * gamma + beta

Sharding: 8 cores = 2 batches x 4 query-row chunks of 512 rows.

Every attention-side matmul runs in fp8e4 with MatmulPerfMode.DoubleRow
(2 stationary planes per matmul, 0.5 cycles/row):
  - q/v projections contract 2 feature-dtiles per matmul,
  - scores use a zero-weight second plane (contraction is only d=64),
  - context pairs two key-tiles as the two planes.
Scale management: query/value are pre-scaled x16 into fp8 on the host,
weights x256; projected q~/v~ land in SBUF as 32*(proj+bias) fp8 (bias added
during the PSUM->SBUF copy via per-partition tensor_scalar); the softmax exp
folds 1/(32*32*sqrt(D)) into its pre-scale; the context ones-column holds
32.0 so the softmax-denominator reciprocal undoes the value scaling for
free.  bv's contribution to fc is folded into an effective bfc on the host.
The fc matmul and its inputs (ctxT bf16, query bf16) stay bf16 for accuracy.

Scheduling: projections run vT -> q -> V so attention pair 0 starts as soon
as the first AllGather rank lands (the tile framework tracks scatter regions
per rank, so scores(kt) only wait for rank kt//4).  The fc query-half runs
during the attention window into an SBUF accumulator (dedicated psum
buffer); the tail is only the fc context-half + LayerNorm.
"""

import numpy as np
import ml_dtypes

import concourse.bass as bass
import concourse.tile as tile
from concourse import bacc, mybir
from concourse.bass_utils import run_bass_kernel_spmd

BF16 = mybir.dt.bfloat16
F32 = mybir.dt.float32
FP8 = mybir.dt.float8e4
NP_BF16 = ml_dtypes.bfloat16
NP_FP8 = ml_dtypes.float8_e4m3

B, S, F, H, D = 2, 2048, 1024, 16, 64
NCORES = 8
RPC = 512            # query rows per core
CHUNKS = 4           # row chunks per batch (= cores per batch group)
KEYS = S             # 2048 keys per batch
NKT = KEYS // 128    # 16 key tiles
NDT = F // 128       # 8 feature tiles
NPAIR = H // 2       # 8 head pairs
EPS = 1e-5

SX = 16.0            # raw query/value scale into fp8
SW = 256.0           # weight scale into fp8
SP = 32.0            # projected q~/v~ scale in fp8
COPY_SCALE = SP / (SX * SW)          # psum -> fp8 copy scale (1/128)
EXP_SCALE = 1.0 / (SP * SP * np.sqrt(D))   # fold 1/sqrt(D) and q/v scales
FP8_MAX = 240.0      # IEEE float8_e4m3 max finite

# AllGather payload layout (fp8 elements):
#   region A: vT chunk as [8 dtile, 128, 512]
#   region B: V  chunk as [8 (keytile,half), 128, 520]  (520 = 8 heads x 65)
A_ELEMS = NDT * 128 * 512           # 524288
B_BLOCK = 128 * 520                 # 66560
B_ELEMS = 8 * B_BLOCK               # 532480

DR = mybir.MatmulPerfMode.DoubleRow

NO_COLL = False
NO_COLL_FREE = False  # timing-only: omit the gather traffic entirely
APPLY_GB = True   # apply gamma/beta in the LN epilogue (skippable when ==1/0)
USE_PB = True     # gpsimd partition_broadcast for the softmax denominators


def _build_kernel():
    nc = bacc.Bacc(
        "TRN2",
        target_bir_lowering=False,
        debug=False,
        enable_asserts=False,
        num_devices=NCORES,
    )

    qT_d = nc.dram_tensor("qT", [F, RPC], BF16, kind="ExternalInput")
    q8_d = nc.dram_tensor("q8", [F, RPC], FP8, kind="ExternalInput")
    v8_d = nc.dram_tensor("v8", [F, RPC], FP8, kind="ExternalInput")
    wq8_d = nc.dram_tensor("wq8", [F, F], FP8, kind="ExternalInput")
    wv8_d = nc.dram_tensor("wv8", [F, F], FP8, kind="ExternalInput")
    bqc_d = nc.dram_tensor("bqc", [128, NDT], F32, kind="ExternalInput")
    bvc_d = nc.dram_tensor("bvc", [128, NDT], F32, kind="ExternalInput")
    wfc_d = nc.dram_tensor("wfc", [2 * F + 1, F], BF16, kind="ExternalInput")
    gam_d = nc.dram_tensor("gam", [1, F], F32, kind="ExternalInput")
    bet_d = nc.dram_tensor("bet", [1, F], F32, kind="ExternalInput")
    out_d = nc.dram_tensor("out", [RPC, F], BF16, kind="ExternalOutput")

    with tile.TileContext(nc) as tc:
        _kernel_body(tc, qT_d, q8_d, v8_d, wq8_d, wv8_d, bqc_d, bvc_d,
                     wfc_d, gam_d, bet_d, out_d)

    nc.compile()
    return nc


def _bcast_row_ap(t, n):
    """AP reading DRAM row tensor [1, n] broadcast to 128 partitions."""
    ap = t.ap()
    return bass.AP(tensor=ap.tensor, offset=ap.offset, ap=[[0, 128], [1, n]])


def _dr(base, plane_stride):
    """Insert the 2-wide DoubleRow plane dim into a [part, n] AP slice."""
    assert len(base.ap) == 2, base.ap
    return bass.AP(tensor=base.tensor, offset=base.offset,
                   ap=[base.ap[0], [plane_stride, 2], base.ap[1]])


def _kernel_body(tc, qT_d, q8_d, v8_d, wq8_d, wv8_d, bqc_d, bvc_d,
                 wfc_d, gam_d, bet_d, out_d):
    nc = tc.nc
    Exp = mybir.ActivationFunctionType.Exp
    Sqrt = mybir.ActivationFunctionType.Sqrt
    Copy = mybir.ActivationFunctionType.Copy
    Square = mybir.ActivationFunctionType.Square
    mult = mybir.AluOpType.mult
    addop = mybir.AluOpType.add
    subop = mybir.AluOpType.subtract

    import contextlib
    ctx = contextlib.ExitStack()
    with ctx:
        persist = ctx.enter_context(tc.tile_pool(name="persist", bufs=1))
        small = ctx.enter_context(tc.tile_pool(name="small", bufs=2))
        bcpool = ctx.enter_context(tc.tile_pool(name="bcpool", bufs=2))
        lnp = ctx.enter_context(tc.tile_pool(name="lnp", bufs=2))
        pps = ctx.enter_context(tc.tile_pool(name="pps", bufs=2, space="PSUM"))
        pctx = ctx.enter_context(tc.tile_pool(name="pctx", bufs=1, space="PSUM"))
        pfc = ctx.enter_context(tc.tile_pool(name="pfc", bufs=1, space="PSUM"))
        dram = ctx.enter_context(tc.tile_pool(name="dram", bufs=1, space="DRAM"))

        # ---- persistent SBUF buffers ----
        vraw8 = persist.tile([128, NDT * RPC], FP8)      # value^T chunk, fp8
        qraw8 = persist.tile([128, NDT * RPC], FP8)      # query^T chunk, fp8
        qTin = persist.tile([128, NDT * RPC], BF16)      # query^T chunk (for fc)
        qT8 = persist.tile([128, NDT * RPC], FP8)        # projected q~, fp8
        wv_all = persist.tile([128, NDT * F], FP8)       # all Wv row-dtiles
        wq_all = persist.tile([128, NDT * F], FP8)       # all Wq row-dtiles
        vT8s = persist.tile([128, NDT * RPC], FP8)       # own v~^T chunk (AG A)
        V8s = persist.tile([128, 8 * 520], FP8)          # own V chunk (AG B)
        vT8f = persist.tile([128, (NDT + 1) * KEYS], FP8)  # v~^T all keys + zero blk
        V8f = persist.tile([128, NKT * 1040], FP8)       # V all keys, 65-col blocks
        pt8 = persist.tile([128, 2 * NKT * 1024], FP8)   # exp(scores^T), fp8, 2 pair-parities
        ctxT = persist.tile([128, NPAIR * RPC], BF16)    # normalized context^T
        wfc_sb = persist.tile([128, 2 * NDT * 1024], BF16)  # all fc weights
        fcq = persist.tile([128, 4 * 1024], F32)         # fc query-half partials
        ones64 = persist.tile([1, 64], BF16)
        ones_bf = persist.tile([1, 128], BF16)
        bqc = persist.tile([128, NDT], F32)
        bvc = persist.tile([128, NDT], F32)
        bfc_sb = persist.tile([1, F], BF16)
        eps_sb = persist.tile([128, 1], F32)
        if APPLY_GB:
            gamma_bc = persist.tile([128, F], F32)
            beta_bc = persist.tile([128, F], F32)

        ag_inA = [dram.tile([A_ELEMS // 2], FP8, name=f"agiA{i}")
                  for i in range(2)]
        ag_outA = [dram.tile([CHUNKS, A_ELEMS // 2], FP8, name=f"agoA{i}")
                   for i in range(2)]
        ag_inB = [dram.tile([B_ELEMS // 2], FP8, name=f"agiB{i}")
                  for i in range(2)]
        ag_outB = [dram.tile([CHUNKS, B_ELEMS // 2], FP8, name=f"agoB{i}")
                   for i in range(2)]

        nc.vector.memset(ones64[:, :], 1.0)
        nc.vector.memset(ones_bf[:, :], 1.0)
        nc.vector.memset(eps_sb[:, :], EPS)
        # zero weight block for the scores' second DoubleRow plane
        nc.gpsimd.memset(vT8f[:, NDT * KEYS:(NDT + 1) * KEYS], 0.0)
        # ones columns staged in V8s (=SP so the denominator reciprocal undoes
        # the V scale); the gather then carries them into every rank's V8f
        nc.gpsimd.memset(
            V8s[:, :].rearrange("p (b h e) -> p b h e", b=8, e=65)[:, :, :, 64:65],
            SP)

        nc.sync.dma_start(out=bvc[:, :], in_=bvc_d[0:128, :])
        nc.sync.dma_start(out=bqc[:, :], in_=bqc_d[0:128, :])

        # batched input loads, ordered so the vT pass starts earliest
        def load_w_half(dst, w_d, hf):
            nc.sync.dma_start(
                out=dst[:, :].rearrange("p (c f) -> p c f", c=NDT)[
                    :, hf * 4:(hf + 1) * 4, :],
                in_=w_d[hf * 512:(hf + 1) * 512, :].rearrange(
                    "(c p) f -> p c f", p=128))

        def load_x_half(dst3, x_d, hf):
            nc.sync.dma_start(
                out=dst3[:, hf * 4:(hf + 1) * 4, :],
                in_=x_d[hf * 512:(hf + 1) * 512, :].rearrange(
                    "(c p) n -> p c n", p=128))

        vraw3 = vraw8[:, :].rearrange("p (c n) -> p c n", c=NDT)
        qraw3 = qraw8[:, :].rearrange("p (c n) -> p c n", c=NDT)
        load_w_half(wv_all, wv8_d, 0)
        load_x_half(vraw3, v8_d, 0)
        load_w_half(wv_all, wv8_d, 1)
        load_x_half(vraw3, v8_d, 1)
        load_w_half(wq_all, wq8_d, 0)
        load_x_half(qraw3, q8_d, 0)
        load_w_half(wq_all, wq8_d, 1)
        load_x_half(qraw3, q8_d, 1)

        wv3 = wv_all[:, :].rearrange("p (c f) -> p c f", c=NDT)
        wq3 = wq_all[:, :].rearrange("p (c f) -> p c f", c=NDT)

        def alloc_octet(nm):
            pss = []
            for i in range(2):
                big = pps.tile([128, 1024], F32, tag="ps", name=f"{nm}{i}")
                pss.append(big[:, 0:512])
                pss.append(big[:, 512:1024])
            big = pctx.tile([128, 1024], F32, tag="cps", name=f"{nm}c")
            pss.append(big[:, 0:512])
            pss.append(big[:, 512:1024])
            big = pfc.tile([128, 1024], F32, tag="pfc", name=f"{nm}f")
            pss.append(big[:, 0:512])
            pss.append(big[:, 512:1024])
            return pss

        def proj_matmuls(nm, w3v, x3):
            pss = alloc_octet(nm)
            for c in range(4):
                for m in range(NDT):
                    nc.tensor.matmul(pss[m][:, :],
                                     w3v[:, 2 * c:2 * c + 2, m * 128:(m + 1) * 128],
                                     x3[:, 2 * c:2 * c + 2, :],
                                     start=(c == 0), stop=(c == 3),
                                     perf_mode=DR)
            return pss

        def proj_copies(pss, dst, bias, ms):
            with nc.allow_low_precision(reason="fp8 attention operands"):
                for m in ms:
                    nc.vector.tensor_scalar(dst[:, m * RPC:(m + 1) * RPC],
                                            pss[m][:, :], COPY_SCALE,
                                            bias[:, m:m + 1], op0=mult, op1=addop)

        def all_gather(in_ap, out_ap):
            if NO_COLL_FREE:
                nc.sync.dma_start(out=out_ap[0], in_=in_ap)
            elif NO_COLL:
                for r in range(CHUNKS):
                    nc.sync.dma_start(out=out_ap[r], in_=in_ap)
            else:
                nc.gpsimd.collective_compute(
                    "AllGather",
                    mybir.AluOpType.bypass,
                    replica_groups=[[0, 1, 2, 3], [4, 5, 6, 7]],
                    ins=[in_ap],
                    outs=[out_ap],
                )

        def gather_A(sub):
            # sub 0: dtiles 0-3, sub 1: dtiles 4-7
            nc.sync.dma_start(
                out=ag_inA[sub][:].rearrange("(t p n) -> p t n", p=128, t=4),
                in_=vT8s[:, :].rearrange("p (t n) -> p t n", t=NDT)[
                    :, sub * 4:(sub + 1) * 4, :])
            all_gather(ag_inA[sub][:], ag_outA[sub][:, :])
            for r in range(CHUNKS):
                nc.gpsimd.dma_start(
                    out=vT8f[:, :].rearrange("p (t n) -> p t n", t=NDT + 1)[
                        :, sub * 4:(sub + 1) * 4, r * RPC:(r + 1) * RPC],
                    in_=ag_outA[sub][r, :].rearrange(
                        "(t p n) -> p t n", p=128, t=4))

        def gather_B(sub):
            # sub 0: key-tiles {4r, 4r+1}, sub 1: {4r+2, 4r+3} of every rank
            nc.sync.dma_start(
                out=ag_inB[sub][:].rearrange("(b p n) -> p b n", p=128, b=4),
                in_=V8s[:, :].rearrange("p (b n) -> p b n", b=8)[
                    :, sub * 4:(sub + 1) * 4, :])
            all_gather(ag_inB[sub][:], ag_outB[sub][:, :])
            for r in range(CHUNKS):
                nc.gpsimd.dma_start(
                    out=V8f[:, :].rearrange("p (k h c) -> p k h c", k=NKT, h=2)[
                        :, r * 4 + sub * 2:r * 4 + sub * 2 + 2, :, :],
                    in_=ag_outB[sub][r, :].rearrange(
                        "(k h p n) -> p k h n", p=128, k=2, h=2))

        # ---- v~^T projection; sub-gather A while q projects ----
        pssT = proj_matmuls("vt", wv3, vraw3)
        proj_copies(pssT, vT8s, bvc, range(0, 4))
        gather_A(0)

        # ---- q~ projection ----
        pssQ = proj_matmuls("qp", wq3, qraw3)
        proj_copies(pssQ, qT8, bqc, range(0, 2))
        proj_copies(pssT, vT8s, bvc, range(4, 8))
        proj_copies(pssQ, qT8, bqc, range(2, 8))

        # ---- V projection in two half-octets on the cps/pfc psums only,
        # so the attention ps ring is never gated on the V chain ----
        def v_half(sub):
            pss = []
            big = pctx.tile([128, 1024], F32, tag="cps", name=f"vv{sub}c")
            pss.append(big[:, 0:512])
            pss.append(big[:, 512:1024])
            big = pfc.tile([128, 1024], F32, tag="pfc", name=f"vv{sub}f")
            pss.append(big[:, 0:512])
            pss.append(big[:, 512:1024])
            for c in range(4):
                for t in (2 * sub, 2 * sub + 1):
                    for half in range(2):
                        nc.tensor.matmul(
                            pss[(t - 2 * sub) * 2 + half][:, :],
                            vraw3[:, 2 * c:2 * c + 2, t * 128:(t + 1) * 128],
                            wv3[:, 2 * c:2 * c + 2, half * 512:(half + 1) * 512],
                            start=(c == 0), stop=(c == 3), perf_mode=DR)
            with nc.allow_low_precision(reason="fp8 attention operands"):
                for t in (2 * sub, 2 * sub + 1):
                    for half in range(2):
                        b = t * 2 + half
                        nc.vector.tensor_scalar(
                            V8s[:, b * 520:(b + 1) * 520].rearrange(
                                "p (h e) -> p h e", e=65)[:, :, 0:64],
                            pss[(t - 2 * sub) * 2 + half][:, :].rearrange(
                                "p (h d) -> p h d", d=64),
                            COPY_SCALE, None, op0=mult)
            gather_B(sub)

        v_half(0)
        v_half(1)
        # A2 feeds only pairs 4-7; gate it behind the B gathers and the fc
        # weight loads so those own the DMA queue first (bypass rewrites one
        # byte of the stage source with its own value, adding only the read
        # edge on the last wfc block)
        nc.vector.tensor_tensor(vT8s[0:1, 4 * RPC:4 * RPC + 1],
                                vT8s[0:1, 4 * RPC:4 * RPC + 1],
                                wfc_sb[0:1, 15 * 1024:15 * 1024 + 1],
                                op=mybir.AluOpType.bypass)
        gather_A(1)

        # ---- attention, one head pair at a time ----
        def normalize_pair(p, cps):
            bcs = bcpool.tile([64, 1024], F32, tag="bcs")
            if USE_PB:
                rec = small.tile([1, 1024], F32, tag="rec")
                nc.vector.reciprocal(rec[:, :], cps[64:65, :])
                nc.gpsimd.partition_broadcast(bcs[:, :], rec[:, :], channels=64)
            else:
                rec = small.tile([1, 1024], BF16, tag="rec")
                with nc.allow_low_precision(reason="softmax denom recip bf16"):
                    nc.vector.reciprocal(rec[:, :], cps[64:65, :])
                bc = pps.tile([128, 1024], F32, tag="ps", name="bc")
                nc.tensor.matmul(bc[0:64, 0:512], ones64[:, :], rec[:, 0:512],
                                 start=True, stop=True)
                nc.tensor.matmul(bc[0:64, 512:1024], ones64[:, :],
                                 rec[:, 512:1024], start=True, stop=True)
                nc.vector.tensor_copy(bcs[:, :], bc[0:64, :])
            nc.vector.tensor_tensor(
                ctxT[0:64, p * RPC:(p + 1) * RPC],
                cps[0:64, 0:512], bcs[:, 0:512], op=mult)
            nc.vector.tensor_tensor(
                ctxT[64:128, p * RPC:(p + 1) * RPC],
                cps[0:64, 512:1024], bcs[:, 512:1024], op=mult)

        def fc_query_group(m):
            # accumulate the query half of fc for row-tile m into fcq (SBUF)
            pf = pfc.tile([128, 1024], F32, tag="pfc", name="pf")
            for kc in range(NDT, 2 * NDT):
                cblk = (kc - NDT) * RPC
                wcol = kc * 1024
                for n in range(2):
                    nc.tensor.matmul(
                        pf[:, n * 512:(n + 1) * 512],
                        qTin[:, cblk + m * 128:cblk + (m + 1) * 128],
                        wfc_sb[:, wcol + n * 512:wcol + (n + 1) * 512],
                        start=(kc == NDT), stop=(kc == 2 * NDT - 1))
            nc.vector.tensor_copy(fcq[:, m * 1024:(m + 1) * 1024], pf[:, :])

        def ctx_block(cp, kt):
            # context^T for pair cp over the (kt-1, kt) key-tile pair
            par = (cp % 2) * NKT * 1024
            cps = ctx_cps[cp]
            vcol = (kt - 1) * 1040
            pcol = par + (kt - 1) * 1024
            nc.tensor.matmul(
                cps[:, 0:512],
                _dr(V8f[:, vcol + (2 * cp) * 65:vcol + (2 * cp) * 65 + 65],
                    1040),
                _dr(pt8[:, pcol:pcol + 512], 1024),
                start=(kt == 1), stop=(kt == NKT - 1), perf_mode=DR)
            nc.tensor.matmul(
                cps[:, 512:1024],
                _dr(V8f[:, vcol + (2 * cp + 1) * 65:vcol + (2 * cp + 1) * 65 + 65],
                    1040),
                _dr(pt8[:, pcol + 512:pcol + 1024], 1024),
                start=(kt == 1), stop=(kt == NKT - 1), perf_mode=DR)

        ctx_cps = {}

        def attn_pair(p):
            # scores/exp for pair p; ctx matmuls lag one pair so they never
            # park in the PE wait queue before the V gather lands
            par = (p % 2) * NKT * 1024
            zoff = NDT * KEYS  # zero block start in vT8f
            for kt in range(NKT):
                ps = pps.tile([128, 1024], F32, tag="ps")
                col = p * KEYS + kt * 128
                nc.tensor.matmul(
                    ps[:, 0:512],
                    _dr(vT8f[0:64, col:col + 128], zoff - p * KEYS),
                    _dr(qT8[0:64, p * RPC:(p + 1) * RPC], 0),
                    start=True, stop=True, perf_mode=DR)
                nc.tensor.matmul(
                    ps[:, 512:1024],
                    _dr(vT8f[64:128, col:col + 128], zoff - p * KEYS),
                    _dr(qT8[64:128, p * RPC:(p + 1) * RPC], 0),
                    start=True, stop=True, perf_mode=DR)
                nc.scalar.activation(pt8[:, par + kt * 1024:par + (kt + 1) * 1024],
                                     ps[:, :], Exp, scale=EXP_SCALE)
                if p >= 1:
                    if kt == 1:
                        if p >= 2:
                            normalize_pair(p - 2, ctx_cps[p - 2])
                        ctx_cps[p - 1] = pctx.tile([65, 1024], F32, tag="cps",
                                                   name="cps")
                        if p == NPAIR - 1:
                            # last pair's own ctx runs un-lagged on the pfc
                            # ring (V8f has long been gathered by now)
                            ctx_cps[p] = pfc.tile([65, 1024], F32, tag="pfc",
                                                  name="cps7")
                    if kt % 2 == 1:
                        ctx_block(p - 1, kt)
                        if p == NPAIR - 1:
                            ctx_block(p, kt)

        def bulk_loads():
            # WAR gate: a dummy write into each destination reads a region
            # the last sub-gather's scatter writes, so these big loads only
            # hit the DMA queue after the startup gather chain is through.
            gate = V8f[0:1, 14 * 1040:14 * 1040 + 1]  # scatter_B(1) rank 3
            with nc.allow_low_precision(reason="scheduling gate dummies"):
                nc.vector.tensor_copy(qTin[0:1, 0:1], gate)
                for qt in range(4):
                    nc.vector.tensor_copy(
                        wfc_sb[0:1, qt * 4 * 1024:qt * 4 * 1024 + 1], gate)
            nc.sync.dma_start(
                out=qTin[:, :].rearrange("p (c n) -> p c n", c=NDT),
                in_=qT_d[:, :].rearrange("(c p) n -> p c n", p=128))
            for qt in range(4):
                nc.sync.dma_start(
                    out=wfc_sb[:, :].rearrange("p (c f) -> p c f", c=2 * NDT)[
                        :, qt * 4:(qt + 1) * 4, :],
                    in_=wfc_d[qt * 512:(qt + 1) * 512, :].rearrange(
                        "(c p) f -> p c f", p=128))
            nc.sync.dma_start(out=bfc_sb[:, :],
                              in_=wfc_d[2 * F:2 * F + 1, :])
            if APPLY_GB:
                nc.sync.dma_start(out=gamma_bc[:, :],
                                  in_=_bcast_row_ap(gam_d, F))
                nc.sync.dma_start(out=beta_bc[:, :],
                                  in_=_bcast_row_ap(bet_d, F))

        def fc_ctx_partial(m):
            # fold the kc0-3 context half of fc for row-tile m into fcq
            pf = pfc.tile([128, 1024], F32, tag="pfc", name="pcp")
            for kc in range(4):
                cblk = kc * RPC
                wcol = kc * 1024
                for n in range(2):
                    nc.tensor.matmul(
                        pf[:, n * 512:(n + 1) * 512],
                        ctxT[:, cblk + m * 128:cblk + (m + 1) * 128],
                        wfc_sb[:, wcol + n * 512:wcol + (n + 1) * 512],
                        start=(kc == 0), stop=(kc == 3))
            nc.vector.tensor_tensor(fcq[:, m * 1024:(m + 1) * 1024],
                                    fcq[:, m * 1024:(m + 1) * 1024],
                                    pf[:, :], op=addop)

        for p in range(NPAIR):
            attn_pair(p)
            if p == 0:
                bulk_loads()
            if 2 <= p <= 5:
                fc_query_group(p - 2)
            if p == 5:
                fc_ctx_partial(0)
                fc_ctx_partial(1)
            if p == 6:
                fc_ctx_partial(2)
                fc_ctx_partial(3)
        # trailer: only the last pair's normalize remains
        normalize_pair(NPAIR - 1, ctx_cps[NPAIR - 1])

        # ---- fc context-half tail (kc4-7) + LayerNorm ----
        # kc4-6 for every row-tile first (their ctxT pairs are normalized
        # well before the attention trailer); kc7 + bias + the LN chain per
        # row-tile after, so the last normalize overlaps fc matmuls.
        pssm = []
        bigA = pps.tile([128, 1024], F32, tag="ps", name="fcA")
        bigB = pctx.tile([128, 1024], F32, tag="cps", name="fcB")
        bigA2 = pps.tile([128, 1024], F32, tag="ps", name="fcA2")
        bigB2 = pfc.tile([128, 1024], F32, tag="pfc", name="fcB2")
        for big in (bigA, bigB, bigA2, bigB2):
            pssm.append(big)
        for m in range(4):
            for kc in range(4, 7):
                cblk = kc * RPC
                wcol = kc * 1024
                for n in range(2):
                    nc.tensor.matmul(
                        pssm[m][:, n * 512:(n + 1) * 512],
                        ctxT[:, cblk + m * 128:cblk + (m + 1) * 128],
                        wfc_sb[:, wcol + n * 512:wcol + (n + 1) * 512],
                        start=(kc == 4), stop=False)
        for m in range(4):
            big = pssm[m]
            for n in range(2):
                nc.tensor.matmul(
                    big[:, n * 512:(n + 1) * 512],
                    ctxT[:, 7 * RPC + m * 128:7 * RPC + (m + 1) * 128],
                    wfc_sb[:, 7 * 1024 + n * 512:7 * 1024 + (n + 1) * 512],
                    start=False, stop=False)
                nc.tensor.matmul(big[:, n * 512:(n + 1) * 512],
                                 ones_bf[:, :],
                                 bfc_sb[:, n * 512:(n + 1) * 512],
                                 start=False, stop=True)
            tsb = lnp.tile([128, 1024], F32, tag="tsb", name="tsb")
            ssum = small.tile([128, 2], F32, tag="ssum", name="ssum")
            for n in range(2):
                # fused: tsb = psum + fcq, ssum[n] = row-sum(tsb half)
                nc.vector.tensor_tensor_reduce(
                    out=tsb[:, n * 512:(n + 1) * 512],
                    in0=big[:, n * 512:(n + 1) * 512],
                    in1=fcq[:, m * 1024 + n * 512:m * 1024 + (n + 1) * 512],
                    scale=1.0, scalar=0.0, op0=addop, op1=addop,
                    accum_out=ssum[:, n:n + 1])
            dump2 = lnp.tile([128, 1024], F32, tag="dump", name="dump2")
            sqs = small.tile([128, 1], F32, tag="sqs", name="sqs")
            nc.scalar.activation(dump2[:, :], tsb[:, :], Square,
                                 accum_out=sqs[:, :])
            mean = small.tile([128, 1], F32, tag="mean", name="mean")
            nc.vector.tensor_scalar(mean[:, :], ssum[:, 0:1], ssum[:, 1:2],
                                    1.0 / F, op0=addop, op1=mult)
            ex2 = small.tile([128, 1], F32, tag="ex2", name="ex2")
            nc.vector.tensor_scalar(ex2[:, :], sqs[:, :], 1.0 / F, None,
                                    op0=mult)
            msq = small.tile([128, 1], F32, tag="msq", name="msq")
            nc.vector.tensor_tensor(msq[:, :], mean[:, :], mean[:, :], op=mult)
            var = small.tile([128, 1], F32, tag="var", name="var")
            nc.vector.tensor_tensor(var[:, :], ex2[:, :], msq[:, :], op=subop)
            sd = small.tile([128, 1], F32, tag="sd", name="sd")
            nc.scalar.activation(sd[:, :], var[:, :], Sqrt, bias=eps_sb[:, :])
            rstd = small.tile([128, 1], F32, tag="rstd", name="rstd")
            nc.vector.reciprocal(rstd[:, :], sd[:, :])
            nmr = small.tile([128, 1], F32, tag="nmr", name="nmr")
            nc.vector.tensor_scalar(nmr[:, :], mean[:, :], rstd[:, :], -1.0,
                                    op0=mult, op1=mult)
            outt = lnp.tile([128, F], BF16, tag="outt", name="outt")
            for n in range(2):
                sl = slice(n * 512, (n + 1) * 512)
                if APPLY_GB:
                    t1 = lnp.tile([128, 512], F32, tag="t1", name="t1")
                    nc.vector.tensor_scalar(t1[:, :], tsb[:, sl],
                                            rstd[:, :], nmr[:, :],
                                            op0=mult, op1=addop)
                    t2 = lnp.tile([128, 512], F32, tag="t1", name="t2")
                    nc.vector.tensor_tensor(t2[:, :], t1[:, :],
                                            gamma_bc[:, sl], op=mult)
                    with nc.allow_low_precision(reason="bf16 output"):
                        nc.vector.tensor_tensor(outt[:, sl], t2[:, :],
                                                beta_bc[:, sl], op=addop)
                else:
                    with nc.allow_low_precision(reason="bf16 output"):
                        nc.vector.tensor_scalar(outt[:, sl], tsb[:, sl],
                                                rstd[:, :], nmr[:, :],
                                                op0=mult, op1=addop)
                nc.sync.dma_start(out=out_d[m * 128:(m + 1) * 128, sl],
                                  in_=outt[:, sl])


_NC_CACHE = {}


def _get_nc():
    key = (APPLY_GB, NO_COLL, NO_COLL_FREE, USE_PB)
    if key not in _NC_CACHE:
        _NC_CACHE[key] = _build_kernel()
    return _NC_CACHE[key]


def _to_fp8(x):
    return np.clip(x, -FP8_MAX, FP8_MAX).astype(NP_FP8)


def _prep_inputs(query, value, Wq, bq, Wv, bv, Wfc, bfc, gamma, beta):
    wq8 = np.ascontiguousarray(_to_fp8(Wq * SW))
    wv8 = np.ascontiguousarray(_to_fp8(Wv * SW))
    bqc = np.ascontiguousarray((bq * SP).reshape(NDT, 128).T).astype(np.float32)
    bvc = np.ascontiguousarray((bv * SP).reshape(NDT, 128).T).astype(np.float32)
    # fold bv's contribution through the context half of Wfc into the bias
    bfc_eff = bfc + bv @ Wfc[:F, :]
    wfc_ext = np.ascontiguousarray(
        np.concatenate([Wfc, bfc_eff[None, :]], axis=0)).astype(NP_BF16)
    gam = np.ascontiguousarray(gamma[None, :]).astype(np.float32)
    bet = np.ascontiguousarray(beta[None, :]).astype(np.float32)

    in_maps = []
    for c in range(NCORES):
        b, r = c // CHUNKS, (c % CHUNKS) * RPC
        qT = np.ascontiguousarray(query[b, r:r + RPC, :].T)
        vT = np.ascontiguousarray(value[b, r:r + RPC, :].T)
        in_maps.append({
            "qT": qT.astype(NP_BF16),
            "q8": _to_fp8(qT * SX),
            "v8": _to_fp8(vT * SX),
            "wq8": wq8, "wv8": wv8, "bqc": bqc, "bvc": bvc,
            "wfc": wfc_ext, "gam": gam, "bet": bet,
        })
    return in_maps


def run_on_hw(in_maps, **kwargs):
    nc = _get_nc()
    return run_bass_kernel_spmd(nc, in_maps, list(range(NCORES)), **kwargs)


def kernel(query, value, Wq, bq, Wv, bv, Wfc, bfc, gamma, beta):
    global APPLY_GB
    APPLY_GB = not (np.all(np.asarray(gamma, np.float32) == 1.0)
                    and np.all(np.asarray(beta, np.float32) == 0.0))
    query = np.asarray(query, dtype=np.float32)
    value = np.asarray(value, dtype=np.float32)
    in_maps = _prep_inputs(query, value,
                           np.asarray(Wq, np.float32), np.asarray(bq, np.float32),
                           np.asarray(Wv, np.float32), np.asarray(bv, np.float32),
                           np.asarray(Wfc, np.float32), np.asarray(bfc, np.float32),
                           np.asarray(gamma, np.float32), np.asarray(beta, np.float32))
    res = run_on_hw(in_maps)
    out = np.empty((B, S, F), np.float32)
    for c in range(NCORES):
        b, r = c // CHUNKS, (c % CHUNKS) * RPC
        out[b, r:r + RPC, :] = np.asarray(res.results[c]["out"],
                                          dtype=np.float32)
    return out
